# revision 1
# baseline (speedup 1.0000x reference)
"""DeepseekV2 MLA attention prefill kernel for 8 Trainium2 NeuronCores.

Sharding: 2-way data-parallel over batch x 4-way tensor-parallel over heads
(4 heads per core).  The q down-projection + RMSNorm is computed on an S/4
slice per core inside each batch group and exchanged with one in-group
AllGather (1.5MB payload); the cheap compressed-KV path is replicated at
full S on every core and computed *while the gather is in flight*, hiding
the collective's ~75us fixed latency.  Per-head up-projections, attention
and the output projection are computed locally; the o_proj partial sums are
reduced on the host during unsharding.

Layouts: activations are feature-major ([D, S]) throughout, attention scores
are computed transposed ([s_k, s_q]) so the PV matmul needs no transposes.
RoPE is applied via host-side permuted/sign-folded weight columns, so the
device only does two elementwise multiplies and an add per rope tensor.
Matmuls run in bf16 (full PE rate) with fp32 PSUM accumulation; softmax and
RMS statistics stay fp32.  Softmax row sums for the 4 heads of one seq chunk
accumulate into separate partition rows of one PSUM bank so a single
reciprocal serves all four heads.
"""
import sys
sys.path.insert(0, "/opt/trn_rl_repo")

import math
import numpy as np
import ml_dtypes

import concourse.bass as bass
import concourse.tile as tile
from concourse import bacc, mybir
from concourse.bass_utils import run_bass_kernel_spmd

# ---- problem constants (hardcoded; kernel.py must be self-contained) ----
B, S, HID, H = 2, 2048, 2048, 16
Q_LORA, KV_LORA = 1536, 512
D_NOPE, D_ROPE, D_V = 128, 64, 128
D_Q = D_NOPE + D_ROPE
EPS = 1e-6
ROPE_THETA = 10000.0
N_CORES = 8
HPC = 4                      # heads per core
GROUPS = [[0, 1, 2, 3], [4, 5, 6, 7]]

PLAN_B = True                # True: gather q_norm; False: replicate q too
G_ROWS = Q_LORA              # gather payload rows (q_norm only)

F32 = mybir.dt.float32
BF16 = mybir.dt.bfloat16
MM_DT = BF16                 # matmul input dtype

SCALE = 1.0 / math.sqrt(D_Q)

_CACHE = {}


# ---------------------------------------------------------------- builder --
def build_kernel(plan_b=PLAN_B, mm_dt=MM_DT):
    s_loc = S // 4 if plan_b else S

    nc = bacc.Bacc("TRN2", target_bir_lowering=False, debug=False,
                   num_devices=N_CORES)

    # inputs (bf16 weights/activations prepared on host)
    xt = nc.dram_tensor("xt", [HID, S], mm_dt, kind="ExternalInput")
    xt_loc = nc.dram_tensor("xt_loc", [HID, s_loc], mm_dt, kind="ExternalInput")
    wdq = nc.dram_tensor("wdq", [HID, Q_LORA], mm_dt, kind="ExternalInput")
    wuq = nc.dram_tensor("wuq", [Q_LORA, HPC * 256], mm_dt, kind="ExternalInput")
    wkva = nc.dram_tensor("wkva", [HID, KV_LORA + 2 * D_ROPE], mm_dt, kind="ExternalInput")
    wkvb = nc.dram_tensor("wkvb", [KV_LORA, HPC, 256], mm_dt, kind="ExternalInput")
    ow = nc.dram_tensor("ow", [HPC, D_V, HID], mm_dt, kind="ExternalInput")
    cos_f = nc.dram_tensor("cos_f", [D_ROPE, S], mm_dt, kind="ExternalInput")
    sin_f = nc.dram_tensor("sin_f", [D_ROPE, S], mm_dt, kind="ExternalInput")
    masks = nc.dram_tensor("masks", [4, 128, 512], mm_dt, kind="ExternalInput")
    out_t = nc.dram_tensor("out_t", [HID, S], F32, kind="ExternalOutput")

    with tile.TileContext(nc) as tc:
        import contextlib
        ctx = contextlib.ExitStack()
        with ctx:
            persist = ctx.enter_context(tc.tile_pool(name="persist", bufs=1))
            wpool = ctx.enter_context(tc.tile_pool(name="wpool", bufs=3))
            spool = ctx.enter_context(tc.tile_pool(name="spool", bufs=2))
            xpool = ctx.enter_context(tc.tile_pool(name="xpool", bufs=3))
            ppool = ctx.enter_context(tc.tile_pool(name="ppool", bufs=2, space="PSUM"))
            pscore = ctx.enter_context(tc.tile_pool(name="pscore", bufs=3, space="PSUM"))
            pctx = ctx.enter_context(tc.tile_pool(name="pctx", bufs=1, space="PSUM"))
            psums = ctx.enter_context(tc.tile_pool(name="psums", bufs=1, space="PSUM"))
            dram = ctx.enter_context(tc.tile_pool(name="dram", bufs=1, space="DRAM"))

            # ---- constants ----
            ones_sb = persist.tile([128, 1], mm_dt, tag="ones")
            nc.vector.memset(ones_sb, 1.0)
            eps_sb = persist.tile([1, 1], F32, tag="eps")
            nc.vector.memset(eps_sb, EPS)
            mask_sb = persist.tile([128, 4, 512], mm_dt, tag="masks")
            nc.sync.dma_start(out=mask_sb, in_=masks.ap().rearrange("d p c -> p d c"))
            cosf_sb = persist.tile([D_ROPE, 4, 512], mm_dt, tag="cosf")
            sinf_sb = persist.tile([D_ROPE, 4, 512], mm_dt, tag="sinf")
            nc.sync.dma_start(out=cosf_sb, in_=cos_f.ap().rearrange("d (c n) -> d c n", c=4))
            nc.sync.dma_start(out=sinf_sb, in_=sin_f.ap().rearrange("d (c n) -> d c n", c=4))

            # gather buffers (DRAM)
            g_in = dram.tile([G_ROWS, s_loc], mm_dt)
            if plan_b:
                g_out = dram.tile([4 * G_ROWS, 512], mm_dt)
            else:
                g_out = g_in

            def g_read(row0, nrows, sqc):
                if plan_b:
                    return g_out[sqc * G_ROWS + row0: sqc * G_ROWS + row0 + nrows, :]
                return g_out[row0: row0 + nrows, sqc * 512:(sqc + 1) * 512]

            def rms_proj(x_dram, w_dram, m_chunks, norm_mchunks, inv_n, consume,
                         nch_list, rope_nch_sb=None):
                """Feature-major projection of x with w + RMS-normalize the
                first norm_mchunks chunks.  consume(m_or_'rope', nch, bf16_tile)
                receives normalized (or rope-combined) output tiles."""
                for nch in nch_list:
                    raw = []
                    ssq = psums.tile([1, 512], F32, tag="p_sum", name="ssq")
                    mgs = [list(range(g, min(g + 4, m_chunks)))
                           for g in range(0, m_chunks, 4)]
                    for mg in mgs:
                        accs = {m: (ppool.tile([128, 512], F32, tag="p_a", name="acc_a")
                                    if j < 2 else
                                    pscore.tile([128, 512], F32, tag="p_sc", name="acc_b"))
                                for j, m in enumerate(mg)}
                        for k in range(HID // 128):
                            wt = wpool.tile([128, 128 * len(mg)], mm_dt, tag="w_s1")
                            nc.sync.dma_start(
                                out=wt,
                                in_=w_dram.ap()[k * 128:(k + 1) * 128,
                                                mg[0] * 128: mg[0] * 128 + 128 * len(mg)])
                            xtt = xpool.tile([128, 512], mm_dt, tag="xt_s")
                            nc.sync.dma_start(
                                out=xtt,
                                in_=x_dram.ap()[k * 128:(k + 1) * 128,
                                                nch * 512:(nch + 1) * 512])
                            for j, m in enumerate(mg):
                                nc.tensor.matmul(
                                    accs[m], wt[:, j * 128:(j + 1) * 128], xtt,
                                    start=(k == 0), stop=(k == HID // 128 - 1))
                        for m in mg:
                            if m < norm_mchunks:
                                sq = spool.tile([128, 512], mm_dt, tag="sq")
                                nc.scalar.activation(out=sq, in_=accs[m],
                                                     func=mybir.ActivationFunctionType.Square)
                                nc.tensor.matmul(ssq, ones_sb, sq,
                                                 start=(m == 0), stop=(m == norm_mchunks - 1),
                                                 skip_group_check=True)
                                r = persist.tile([128, 512], mm_dt, tag="raw%d" % m)
                                nc.vector.tensor_copy(r, accs[m])
                                raw.append((m, r))
                            else:   # rope chunk [E(64) | R(64)]
                                cs, sn = rope_nch_sb(nch)
                                t0 = spool.tile([D_ROPE, 512], F32, tag="ropet0")
                                t1 = spool.tile([D_ROPE, 512], F32, tag="ropet1")
                                nc.vector.tensor_tensor(
                                    t0, accs[m][0:D_ROPE, :], cs, mybir.AluOpType.mult)
                                nc.vector.tensor_tensor(
                                    t1, accs[m][D_ROPE:2 * D_ROPE, :], sn,
                                    mybir.AluOpType.mult)
                                pe = spool.tile([D_ROPE, 512], mm_dt, tag="ropeo")
                                nc.vector.tensor_tensor(pe, t0, t1, mybir.AluOpType.add)
                                consume("rope", nch, pe)
                    sd = spool.tile([1, 512], F32, tag="sd")
                    nc.scalar.activation(out=sd, in_=ssq,
                                         func=mybir.ActivationFunctionType.Sqrt,
                                         bias=eps_sb, scale=inv_n)
                    rstd = spool.tile([1, 512], F32, tag="rstd")
                    nc.vector.reciprocal(rstd, sd)
                    rstd_bc = spool.tile([128, 512], F32, tag="rstd_bc")
                    nc.gpsimd.partition_broadcast(rstd_bc, rstd)
                    for m, r in raw:
                        o = spool.tile([128, 512], mm_dt, tag="normo")
                        nc.vector.tensor_tensor(o, r, rstd_bc, mybir.AluOpType.mult)
                        consume(m, nch, o)

            # ---- stage 1a: q down-proj + RMSNorm on the local S slice ----
            def q_consume(m, nch, o):
                nc.gpsimd.dma_start(
                    out=g_in[m * 128:(m + 1) * 128, nch * 512:(nch + 1) * 512],
                    in_=o)

            rms_proj(xt_loc, wdq, Q_LORA // 128, Q_LORA // 128, 1.0 / Q_LORA,
                     q_consume, list(range(s_loc // 512)))

            # ---- stage 1b: AllGather q_norm within batch groups ----
            if plan_b:
                nc.gpsimd.collective_compute(
                    "AllGather", mybir.AluOpType.bypass,
                    replica_groups=GROUPS,
                    ins=[g_in.opt()], outs=[g_out.opt()])

            # ---- stage 1c (overlaps gather): compressed KV at full S ----
            ckv_sb = persist.tile([128, KV_LORA // 128, 4, 512], mm_dt, tag="ckv")
            kpe_sb = persist.tile([D_ROPE, 4, 512], mm_dt, tag="kpe")

            def kv_consume(m, nch, o):
                if m == "rope":
                    nc.vector.tensor_copy(kpe_sb[:, nch, :], o)
                else:
                    nc.vector.tensor_copy(ckv_sb[:, m, nch, :], o)

            rms_proj(xt, wkva, 5, KV_LORA // 128, 1.0 / KV_LORA, kv_consume,
                     list(range(4)),
                     rope_nch_sb=lambda nch: (cosf_sb[:, nch, :], sinf_sb[:, nch, :]))

            # ---- stage 2a: decompress KV (full S, local heads) ----
            wkvb_sb = persist.tile([128, KV_LORA // 128, HPC, 256], mm_dt, tag="wkvb")
            nc.sync.dma_start(out=wkvb_sb,
                              in_=wkvb.ap().rearrange("(kc p) h c -> p kc h c", p=128))

            kn_sb = persist.tile([D_NOPE, HPC, 4, 512], mm_dt, tag="kn")
            for h in range(HPC):
                for skc in range(4):
                    acc = ppool.tile([128, 512], F32, tag="p_a", name="acc_kn")
                    for k in range(KV_LORA // 128):
                        nc.tensor.matmul(acc, wkvb_sb[:, k, h, 0:128],
                                         ckv_sb[:, k, skc, :],
                                         start=(k == 0), stop=(k == KV_LORA // 128 - 1))
                    nc.scalar.copy(kn_sb[:, h, skc, :], acc)

            v_sb = persist.tile([128, S // 128, HPC * D_V], mm_dt, tag="v")
            for skt in range(S // 128):
                acc = ppool.tile([128, 512], F32, tag="p_a", name="acc_v")
                for k in range(KV_LORA // 128):
                    nc.tensor.matmul(
                        acc,
                        ckv_sb[:, k, skt // 4, (skt % 4) * 128:(skt % 4) * 128 + 128],
                        wkvb_sb[:, k, :, 128:256],
                        start=(k == 0), stop=(k == KV_LORA // 128 - 1))
                nc.scalar.copy(v_sb[:, skt, :], acc)

            # ---- stage 2b/2c: per-seq-chunk q up-proj + attention ----
            ctx_sb = persist.tile([D_V, HPC, 4, 512], mm_dt, tag="ctx")

            for sqc in range(4):
                qn_t = {}
                qpe_t = {}
                qnorm_t = [spool.tile([128, 512], mm_dt, tag="qn_stream%d" % (k % 4),
                                      name="qnorm_t", bufs=3) for k in range(Q_LORA // 128)]
                for k in range(Q_LORA // 128):
                    nc.sync.dma_start(out=qnorm_t[k], in_=g_read(k * 128, 128, sqc))
                for g2 in range(4):   # one head (nope + rope chunk) per pass
                    accs = [ppool.tile([128, 512], F32, tag="p_a", name="acc_qup")
                            for _ in range(2)]
                    for k in range(Q_LORA // 128):
                        wt = wpool.tile([128, 256], mm_dt, tag="w_uq")
                        nc.sync.dma_start(
                            out=wt,
                            in_=wuq.ap()[k * 128:(k + 1) * 128,
                                         g2 * 256:(g2 + 1) * 256])
                        for j in range(2):
                            nc.tensor.matmul(accs[j], wt[:, j * 128:(j + 1) * 128],
                                             qnorm_t[k],
                                             start=(k == 0), stop=(k == Q_LORA // 128 - 1))
                    h = g2
                    qt = spool.tile([D_NOPE, 512], mm_dt, tag="qn_h%d" % h, bufs=1)
                    nc.scalar.copy(qt, accs[0])
                    qn_t[h] = qt
                    t0 = spool.tile([D_ROPE, 512], F32, tag="qropet0")
                    t1 = spool.tile([D_ROPE, 512], F32, tag="qropet1")
                    nc.vector.tensor_tensor(t0, accs[1][0:D_ROPE, :],
                                            cosf_sb[:, sqc, :], mybir.AluOpType.mult)
                    nc.vector.tensor_tensor(t1, accs[1][D_ROPE:2 * D_ROPE, :],
                                            sinf_sb[:, sqc, :], mybir.AluOpType.mult)
                    qpt = spool.tile([D_ROPE, 512], mm_dt, tag="qpe_h%d" % h, bufs=1)
                    nc.vector.tensor_tensor(qpt, t0, t1, mybir.AluOpType.add)
                    qpe_t[h] = qpt

                n_skt = 4 * (sqc + 1)
                ctx_raw = {}
                recips = {}
                for h in range(HPC):
                    sum_acc = psums.tile([1, 512], F32, tag="p_sum", name="sum_acc")
                    ctx_acc = pctx.tile([D_V, 512], F32, tag="p_ctx")
                    pending = None   # software pipeline: exp tile awaiting sums/PV
                    for skt in range(n_skt):
                        sc = pscore.tile([128, 512], F32, tag="p_sc", name="sc")
                        nc.tensor.matmul(
                            sc, kn_sb[:, h, skt // 4, (skt % 4) * 128:(skt % 4) * 128 + 128],
                            qn_t[h], start=True, stop=False, skip_group_check=True)
                        nc.tensor.matmul(
                            sc, kpe_sb[:, skt // 4, (skt % 4) * 128:(skt % 4) * 128 + 128],
                            qpe_t[h], start=False, stop=True, skip_group_check=True)
                        ex = spool.tile([128, 512], mm_dt, tag="exp%d" % (skt % 3), bufs=2)
                        nc.scalar.activation(out=ex, in_=sc,
                                             func=mybir.ActivationFunctionType.Exp,
                                             scale=SCALE)
                        if skt >= 4 * sqc:
                            nc.vector.tensor_tensor(ex, ex, mask_sb[:, skt - 4 * sqc, :],
                                                    mybir.AluOpType.mult)
                        if pending is not None:
                            pex, pskt = pending
                            nc.tensor.matmul(sum_acc, ones_sb, pex,
                                             start=(pskt == 0), stop=False,
                                             skip_group_check=True)
                            nc.tensor.matmul(ctx_acc, v_sb[:, pskt, h * D_V:(h + 1) * D_V],
                                             pex, start=(pskt == 0), stop=False,
                                             skip_group_check=True)
                        pending = (ex, skt)
                    pex, pskt = pending
                    nc.tensor.matmul(sum_acc, ones_sb, pex,
                                     start=(pskt == 0), stop=True,
                                     skip_group_check=True)
                    nc.tensor.matmul(ctx_acc, v_sb[:, pskt, h * D_V:(h + 1) * D_V],
                                     pex, start=(pskt == 0), stop=True,
                                     skip_group_check=True)
                    # 1/sum via exp(-ln(sum)) on ScalarE (keeps DVE free)
                    ls = spool.tile([1, 512], F32, tag="lsum")
                    nc.scalar.activation(out=ls, in_=sum_acc,
                                         func=mybir.ActivationFunctionType.Ln)
                    rc = spool.tile([1, 512], F32, tag="recip1_%d" % h, bufs=1)
                    nc.scalar.activation(out=rc, in_=ls, scale=-1.0,
                                         func=mybir.ActivationFunctionType.Exp)
                    recips[h] = rc
                    cr = spool.tile([D_V, 512], mm_dt, tag="ctxr%d" % h, bufs=1)
                    nc.scalar.copy(cr, ctx_acc)
                    ctx_raw[h] = cr

                for h in range(HPC):
                    rb = spool.tile([128, 512], F32, tag="recip_bc")
                    nc.gpsimd.partition_broadcast(rb, recips[h])
                    nc.vector.tensor_tensor(ctx_sb[:, h, sqc, :], ctx_raw[h], rb,
                                            mybir.AluOpType.mult)

            # ---- stage 2d: output projection (partial sums over local heads) ----
            ow_sb = persist.tile([D_V, HPC, HID], mm_dt, tag="ow")
            nc.sync.dma_start(out=ow_sb, in_=ow.ap().rearrange("h p c -> p h c"))
            for hidc in range(HID // 128):
                for sqc in range(4):
                    acc = ppool.tile([128, 512], F32, tag="p_a", name="acc_o")
                    for h in range(HPC):
                        nc.tensor.matmul(acc, ow_sb[:, h, hidc * 128:(hidc + 1) * 128],
                                         ctx_sb[:, h, sqc, :],
                                         start=(h == 0), stop=(h == HPC - 1))
                    o = spool.tile([128, 512], F32, tag="oout")
                    nc.scalar.copy(o, acc)
                    nc.gpsimd.dma_start(
                        out=out_t.ap()[hidc * 128:(hidc + 1) * 128,
                                       sqc * 512:(sqc + 1) * 512],
                        in_=o)

    nc.compile()
    return nc


# ------------------------------------------------------------- host side --
def _rope_tables():
    inv_freq = 1.0 / (ROPE_THETA ** (np.arange(0, D_ROPE, 2, dtype=np.float64) / D_ROPE))
    t = np.arange(S, dtype=np.float64)
    freqs = np.outer(t, inv_freq)                    # [S, 32]
    emb = np.concatenate([freqs, freqs], axis=-1)    # [S, 64]
    return (np.cos(emb).astype(np.float32).T.copy(),
            np.sin(emb).astype(np.float32).T.copy())  # [64, S]


_E_PERM = np.concatenate([np.arange(0, D_ROPE, 2), np.arange(1, D_ROPE, 2)])


def _rope_expand(Wpe):
    """[n, 64] rope weight cols -> [n, 128]: [even/odd-reordered | rot-half signed]."""
    Y = Wpe[:, _E_PERM]
    R = np.concatenate([-Y[:, D_ROPE // 2:], Y[:, :D_ROPE // 2]], axis=1)
    return np.concatenate([Y, R], axis=1)


def _prep_inputs(hidden_states, w_dq, q_a_ln_w, w_uq, kv_a_w, kv_a_ln_w, kv_b_w, o_w,
                 plan_b=PLAN_B):
    bf = ml_dtypes.bfloat16
    s_loc = S // 4 if plan_b else S
    cosT, sinT = _rope_tables()

    wuq_eff = (np.asarray(q_a_ln_w)[:, None] * np.asarray(w_uq)).reshape(Q_LORA, H, D_Q)
    head_blocks = []
    for h in range(H):
        head_blocks.append(np.concatenate(
            [wuq_eff[:, h, :D_NOPE], _rope_expand(wuq_eff[:, h, D_NOPE:])], axis=1))
    wuq_x = np.stack(head_blocks, axis=1)            # [1536, 16, 256]

    kv_a = np.asarray(kv_a_w)
    wkva_x = np.concatenate([kv_a[:, :KV_LORA], _rope_expand(kv_a[:, KV_LORA:])],
                            axis=1).astype(bf)       # [2048, 640]
    wkvb_eff = (np.asarray(kv_a_ln_w)[:, None] * np.asarray(kv_b_w)).reshape(KV_LORA, H, 256)
    ow_r = np.asarray(o_w).reshape(H, D_V, HID)

    c_idx = np.arange(512)[None, :]
    r_idx = np.arange(128)[:, None]
    masks = np.stack([(c_idx >= 128 * dd + r_idx) for dd in range(4)]).astype(bf)

    wdq_b = np.asarray(w_dq).astype(bf)
    hs = np.asarray(hidden_states)

    in_maps = []
    for c in range(N_CORES):
        b, hg = c // 4, c % 4
        s0 = 512 * hg if plan_b else 0
        xt_full = np.ascontiguousarray(hs[b].T).astype(bf)
        in_maps.append({
            "xt": xt_full,
            "xt_loc": np.ascontiguousarray(xt_full[:, s0:s0 + s_loc]),
            "wdq": wdq_b,
            "wuq": np.ascontiguousarray(
                wuq_x[:, HPC * hg: HPC * (hg + 1), :].reshape(Q_LORA, HPC * 256)).astype(bf),
            "wkva": wkva_x,
            "wkvb": np.ascontiguousarray(
                wkvb_eff[:, HPC * hg: HPC * (hg + 1), :]).astype(bf),
            "ow": np.ascontiguousarray(ow_r[HPC * hg: HPC * (hg + 1)]).astype(bf),
            "cos_f": cosT.astype(bf),
            "sin_f": sinT.astype(bf),
            "masks": masks,
        })
    return in_maps


def _postprocess(results):
    out = np.empty((B, S, HID), dtype=np.float32)
    for b in range(B):
        acc = results[4 * b]["out_t"].astype(np.float32).copy()
        for c in GROUPS[b][1:]:
            acc += results[c]["out_t"]
        out[b] = acc.T
    return out


def kernel(**inputs):
    key = (PLAN_B, str(MM_DT))
    if key not in _CACHE:
        _CACHE[key] = build_kernel(PLAN_B, MM_DT)
    nc = _CACHE[key]
    in_maps = _prep_inputs(**inputs, plan_b=PLAN_B)
    r = run_bass_kernel_spmd(nc, in_maps, core_ids=list(range(N_CORES)))
    return _postprocess(r.results)



# revision 12
# speedup vs baseline: 1.1042x; 1.1042x over previous
"""DeepseekV2 MLA attention prefill kernel for 8 Trainium2 NeuronCores.

Sharding: 2-way data-parallel over batch x 4-way tensor-parallel over heads
(4 heads per core).  The q down-projection + RMSNorm is computed on an S/4
slice per core inside each batch group and exchanged with one in-group
AllGather; the compressed-KV path is replicated at full S on every core and
computed while the gather is in flight.  Per-head up-projections, attention
and the output projection are computed locally; o_proj partial sums are
reduced on the host during unsharding.

v2 scheduling fixes over v1:
 - the AllGather is issued from the sync engine, so the gpsimd queue (which
   carries the partition_broadcast ops of the kv RMS-norm) is not blocked
   behind the ~90us collective (v1's 63us tensor-engine stall);
 - stage-2 weights live in persistent SBUF tiles DMA'd up front;
 - o_proj runs inside the per-seq-chunk loop so output DMA streams;
 - rope score matmuls for two adjacent key blocks run concurrently in
   disjoint PE row groups (K=64 each, via tile_position);
 - PV and row-sum matmuls restrict their free dim on diagonal blocks;
 - PSUM evacuation copies run on the vector engine; output is bf16.

Layouts: activations are feature-major ([D, S]) throughout, attention scores
are computed transposed ([s_k, s_q]) so the PV matmul needs no transposes.
RoPE is applied via host-side permuted/sign-folded weight columns.  Matmuls
run in bf16 with fp32 PSUM accumulation; softmax and RMS statistics are fp32.
"""
import sys
sys.path.insert(0, "/opt/trn_rl_repo")

import math
import numpy as np
import ml_dtypes

import concourse.bass as bass
import concourse.tile as tile
from concourse import bacc, mybir
from concourse.bass_utils import run_bass_kernel_spmd

# ---- problem constants (hardcoded; kernel.py must be self-contained) ----
B, S, HID, H = 2, 2048, 2048, 16
Q_LORA, KV_LORA = 1536, 512
D_NOPE, D_ROPE, D_V = 128, 64, 128
D_Q = D_NOPE + D_ROPE
EPS = 1e-6
ROPE_THETA = 10000.0
N_CORES = 8
HPC = 4                      # heads per core
GROUPS = [[0, 1, 2, 3], [4, 5, 6, 7]]

KC = HID // 128              # 16
QC = Q_LORA // 128           # 12
VC = KV_LORA // 128          # 4

F32 = mybir.dt.float32
BF16 = mybir.dt.bfloat16
MM_DT = BF16

SCALE = 1.0 / math.sqrt(D_Q)

_CACHE = {}


# ---------------------------------------------------------------- builder --
def build_kernel(mm_dt=MM_DT):
    s_loc = S // 4

    nc = bacc.Bacc("TRN2", target_bir_lowering=False, debug=False,
                   num_devices=N_CORES)

    xt = nc.dram_tensor("xt", [HID, S], mm_dt, kind="ExternalInput")
    xt_loc = nc.dram_tensor("xt_loc", [HID, s_loc], mm_dt, kind="ExternalInput")
    wdq = nc.dram_tensor("wdq", [HID, Q_LORA], mm_dt, kind="ExternalInput")
    wuq = nc.dram_tensor("wuq", [128, QC, HPC * 256], mm_dt, kind="ExternalInput")
    wkva = nc.dram_tensor("wkva", [128, KC, KV_LORA + 2 * D_ROPE], mm_dt,
                          kind="ExternalInput")
    wkvb = nc.dram_tensor("wkvb", [128, VC, HPC, 256], mm_dt, kind="ExternalInput")
    ow = nc.dram_tensor("ow", [D_V, HPC, HID], mm_dt, kind="ExternalInput")
    cos_f = nc.dram_tensor("cos_f", [D_ROPE, S], mm_dt, kind="ExternalInput")
    sin_f = nc.dram_tensor("sin_f", [D_ROPE, S], mm_dt, kind="ExternalInput")
    masks = nc.dram_tensor("masks", [128, 4, 512], mm_dt, kind="ExternalInput")
    out_t = nc.dram_tensor("out_t", [HID, S], mm_dt, kind="ExternalOutput")

    with tile.TileContext(nc) as tc:
        import contextlib
        ctx = contextlib.ExitStack()
        with ctx:
            persist = ctx.enter_context(tc.tile_pool(name="persist", bufs=1))
            wpool = ctx.enter_context(tc.tile_pool(name="wpool", bufs=3))
            spool = ctx.enter_context(tc.tile_pool(name="spool", bufs=2))
            xpool = ctx.enter_context(tc.tile_pool(name="xpool", bufs=3))
            ppool = ctx.enter_context(tc.tile_pool(name="ppool", bufs=2, space="PSUM"))
            pscore = ctx.enter_context(tc.tile_pool(name="pscore", bufs=4, space="PSUM"))
            pctx = ctx.enter_context(tc.tile_pool(name="pctx", bufs=1, space="PSUM"))
            psums = ctx.enter_context(tc.tile_pool(name="psums", bufs=1, space="PSUM"))
            dram = ctx.enter_context(tc.tile_pool(name="dram", bufs=1, space="DRAM"))

            ones_sb = persist.tile([128, 1], mm_dt, tag="ones")
            nc.vector.memset(ones_sb, 1.0)
            eps_sb = persist.tile([1, 1], F32, tag="eps")
            nc.vector.memset(eps_sb, EPS)

            # local xt slice resident in SBUF (q-down reads it 3x)
            xtl_sb = persist.tile([128, KC, 512], mm_dt, tag="xtl")
            for k in range(KC):
                nc.sync.dma_start(out=xtl_sb[:, k, :],
                                  in_=xt_loc.ap()[k * 128:(k + 1) * 128, :])

            # persistent weights on the gpsimd queue (independent of the
            # sync queue that will carry the collective)
            wkva_sb = persist.tile([128, KC, KV_LORA + 2 * D_ROPE], mm_dt, tag="wkva")
            nc.gpsimd.dma_start(out=wkva_sb, in_=wkva.ap())
            wkvb_sb = persist.tile([128, VC, HPC, 256], mm_dt, tag="wkvb")
            nc.gpsimd.dma_start(out=wkvb_sb, in_=wkvb.ap())
            mask_sb = persist.tile([128, 4, 512], mm_dt, tag="masks")
            nc.gpsimd.dma_start(out=mask_sb, in_=masks.ap())
            cosf_sb = persist.tile([D_ROPE, 4, 512], mm_dt, tag="cosf")
            sinf_sb = persist.tile([D_ROPE, 4, 512], mm_dt, tag="sinf")
            nc.gpsimd.dma_start(out=cosf_sb,
                                in_=cos_f.ap().rearrange("d (c n) -> d c n", c=4))
            nc.gpsimd.dma_start(out=sinf_sb,
                                in_=sin_f.ap().rearrange("d (c n) -> d c n", c=4))

            # gather buffers (DRAM): 12 raw q-down chunks + broadcast rstd
            GR = Q_LORA + 128
            g_in = dram.tile([GR, s_loc], mm_dt)
            g_out = dram.tile([4 * GR, 512], mm_dt)

            # ---- stage 1a: q down-proj (raw) + RMS stats on local slice ----
            # The RMS normalization is folded into the q up-projection output
            # after the gather, so the gather ships raw chunks + rstd.
            ssq_q = psums.tile([1, 512], F32, tag="p_sum", name="ssq_q")
            mgs = [list(range(g, g + 4)) for g in range(0, QC, 4)]
            for mg in mgs:
                accs = {m: (ppool.tile([128, 512], F32, tag="p_a", name="acc_a")
                            if j < 2 else
                            pscore.tile([128, 512], F32, tag="p_sc", name="acc_b"))
                        for j, m in enumerate(mg)}
                for k in range(KC):
                    wt = wpool.tile([128, 512], mm_dt, tag="w_s1")
                    nc.sync.dma_start(
                        out=wt,
                        in_=wdq.ap()[k * 128:(k + 1) * 128,
                                     mg[0] * 128: mg[0] * 128 + 512])
                    for j, m in enumerate(mg):
                        nc.tensor.matmul(
                            accs[m], wt[:, j * 128:(j + 1) * 128], xtl_sb[:, k, :],
                            start=(k == 0), stop=(k == KC - 1))
                for m in mg:
                    sq = spool.tile([128, 512], mm_dt, tag="sq")
                    nc.scalar.activation(out=sq, in_=accs[m],
                                         func=mybir.ActivationFunctionType.Square)
                    nc.tensor.matmul(ssq_q, ones_sb, sq,
                                     start=(m == 0), stop=(m == QC - 1),
                                     skip_group_check=True)
                    r = spool.tile([128, 512], mm_dt, tag="qdout%d" % (m % 4))
                    nc.vector.tensor_copy(r, accs[m])
                    nc.sync.dma_start(out=g_in[m * 128:(m + 1) * 128, :], in_=r)
            sd = spool.tile([1, 512], F32, tag="sdn")
            nc.scalar.activation(out=sd, in_=ssq_q,
                                 func=mybir.ActivationFunctionType.Sqrt,
                                 bias=eps_sb, scale=1.0 / Q_LORA)
            rstd = spool.tile([1, 512], F32, tag="rstdn")
            nc.vector.reciprocal(rstd, sd)
            rstd_bc = spool.tile([128, 512], F32, tag="rstd_bc_q", bufs=1)
            nc.gpsimd.partition_broadcast(rstd_bc, rstd)
            rstd_bcb = spool.tile([128, 512], mm_dt, tag="rstd_bcb_q", bufs=1)
            nc.vector.tensor_copy(rstd_bcb, rstd_bc)
            nc.sync.dma_start(out=g_in[Q_LORA:GR, :], in_=rstd_bcb)

            # ---- stage 1b: AllGather q_norm within batch groups ----
            # Issued from the sync engine (via the gpsimd-class method bound
            # to the SP queue): the collective blocks its issuing queue until
            # completion, and the only things behind it on sync are the
            # (gather-dependent) q_norm readbacks.
            nc.gpsimd.collective_compute(
                "AllGather", mybir.AluOpType.bypass,
                replica_groups=GROUPS,
                ins=[g_in.opt()], outs=[g_out.opt()])

            # ---- stage 1c (overlaps gather): compressed KV at full S ----
            ckv_sb = persist.tile([128, VC, 4, 512], mm_dt, tag="ckv")
            # k_pe packed in skt pairs: partitions 0-63 hold even skt blocks,
            # 64-127 odd ones, so two K=64 rope matmuls share one PE pass.
            kpe2_sb = persist.tile([128, 8, 128], mm_dt, tag="kpe2")

            for nch in range(4):
                raw = []
                ssq = psums.tile([1, 512], F32, tag="p_sum", name="ssq_kv")
                accs = {m: (ppool.tile([128, 512], F32, tag="p_a", name="acc_a")
                            if m < 2 else
                            pscore.tile([128, 512], F32, tag="p_sc", name="acc_b"))
                        for m in range(5)}
                for k in range(KC):
                    xtt = xpool.tile([128, 512], mm_dt, tag="xt_s")
                    nc.sync.dma_start(
                        out=xtt,
                        in_=xt.ap()[k * 128:(k + 1) * 128,
                                    nch * 512:(nch + 1) * 512])
                    for m in range(5):
                        nc.tensor.matmul(
                            accs[m], wkva_sb[:, k, m * 128:(m + 1) * 128], xtt,
                            start=(k == 0), stop=(k == KC - 1))
                for m in range(4):
                    sq = spool.tile([128, 512], mm_dt, tag="sq")
                    nc.scalar.activation(out=sq, in_=accs[m],
                                         func=mybir.ActivationFunctionType.Square)
                    nc.tensor.matmul(ssq, ones_sb, sq,
                                     start=(m == 0), stop=(m == 3),
                                     skip_group_check=True)
                    r = spool.tile([128, 512], mm_dt, tag="kvraw%d" % m, bufs=1)
                    nc.vector.tensor_copy(r, accs[m])
                    raw.append((m, r))
                # rope chunk [E(64) | R(64)] -> k_pe, then pair-pack via DMA
                t0 = spool.tile([D_ROPE, 512], mm_dt, tag="ropet0")
                t1 = spool.tile([D_ROPE, 512], mm_dt, tag="ropet1")
                nc.vector.tensor_tensor(t0, accs[4][0:D_ROPE, :],
                                        cosf_sb[:, nch, :], mybir.AluOpType.mult)
                nc.vector.tensor_tensor(t1, accs[4][D_ROPE:2 * D_ROPE, :],
                                        sinf_sb[:, nch, :], mybir.AluOpType.mult)
                pe = spool.tile([D_ROPE, 512], mm_dt, tag="ropeo")
                nc.vector.tensor_tensor(pe, t0, t1, mybir.AluOpType.add)
                for j in range(4):      # skt = 4*nch + j; pair p = skt//2
                    skt = 4 * nch + j
                    half = skt % 2
                    nc.sync.dma_start(
                        out=kpe2_sb[half * 64:(half + 1) * 64, skt // 2, :],
                        in_=pe[:, j * 128:(j + 1) * 128])
                sd2 = spool.tile([1, 512], F32, tag="sdn")
                nc.scalar.activation(out=sd2, in_=ssq,
                                     func=mybir.ActivationFunctionType.Sqrt,
                                     bias=eps_sb, scale=1.0 / KV_LORA)
                rstd2 = spool.tile([1, 512], F32, tag="rstdn")
                nc.vector.reciprocal(rstd2, sd2)
                rstd2_bc = spool.tile([128, 512], F32, tag="rstd_bc_kv")
                nc.gpsimd.partition_broadcast(rstd2_bc, rstd2)
                for m, r in raw:
                    nc.vector.tensor_tensor(ckv_sb[:, m, nch, :], r, rstd2_bc,
                                            mybir.AluOpType.mult)

            # ---- stage 2a: decompress KV (full S, local heads) ----
            kn_sb = persist.tile([D_NOPE, HPC, 4, 512], mm_dt, tag="kn")
            for h in range(HPC):
                for skc in range(4):
                    acc = ppool.tile([128, 512], F32, tag="p_a", name="acc_kn")
                    for k in range(VC):
                        nc.tensor.matmul(acc, wkvb_sb[:, k, h, 0:128],
                                         ckv_sb[:, k, skc, :],
                                         start=(k == 0), stop=(k == VC - 1))
                    nc.vector.tensor_copy(kn_sb[:, h, skc, :], acc)

            v_sb = persist.tile([128, S // 128, HPC * D_V], mm_dt, tag="v")
            for skt in range(S // 128):
                acc = ppool.tile([128, 512], F32, tag="p_a", name="acc_v")
                for k in range(VC):
                    nc.tensor.matmul(
                        acc,
                        ckv_sb[:, k, skt // 4, (skt % 4) * 128:(skt % 4) * 128 + 128],
                        wkvb_sb[:, k, :, 128:256],
                        start=(k == 0), stop=(k == VC - 1))
                nc.vector.tensor_copy(v_sb[:, skt, :], acc)

            # ---- stage 2b/2c/2d: per-seq-chunk q up-proj, attention, o ----
            for sqc in range(4):
                qnorm_t = [spool.tile([128, 512], mm_dt, tag="qn_stream%d" % (k % 4),
                                      name="qnorm_t", bufs=3) for k in range(QC)]
                for k in range(QC):
                    nc.sync.dma_start(
                        out=qnorm_t[k],
                        in_=g_out[sqc * GR + k * 128: sqc * GR + (k + 1) * 128, :])
                rstd_t = spool.tile([128, 512], mm_dt, tag="rstd_t", bufs=2)
                nc.sync.dma_start(out=rstd_t,
                                  in_=g_out[sqc * GR + Q_LORA: (sqc + 1) * GR, :])
                qn_t = {}
                qpe_t = {}
                for h in range(HPC):
                    wts = []
                    for k in range(QC):
                        wt = wpool.tile([128, 256], mm_dt, tag="w_uq")
                        nc.sync.dma_start(
                            out=wt, in_=wuq.ap()[:, k, h * 256:(h + 1) * 256])
                        wts.append(wt)
                    accs = [ppool.tile([128, 512], F32, tag="p_a", name="acc_qup")
                            for _ in range(2)]
                    for k in range(QC):
                        for j in range(2):
                            nc.tensor.matmul(
                                accs[j], wts[k][:, j * 128:(j + 1) * 128],
                                qnorm_t[k],
                                start=(k == 0), stop=(k == QC - 1))
                    # RMS rstd folded in at the q-up output (per-token scale)
                    qt = spool.tile([D_NOPE, 512], mm_dt, tag="qn_h%d" % h, bufs=1)
                    nc.vector.tensor_tensor(qt, accs[0], rstd_t,
                                            mybir.AluOpType.mult)
                    qn_t[h] = qt
                    t0 = spool.tile([D_ROPE, 512], mm_dt, tag="ropet0")
                    t1 = spool.tile([D_ROPE, 512], mm_dt, tag="ropet1")
                    nc.vector.tensor_tensor(t0, accs[1][0:D_ROPE, :],
                                            cosf_sb[:, sqc, :], mybir.AluOpType.mult)
                    nc.vector.tensor_tensor(t1, accs[1][D_ROPE:2 * D_ROPE, :],
                                            sinf_sb[:, sqc, :], mybir.AluOpType.mult)
                    t2 = spool.tile([D_ROPE, 512], mm_dt, tag="ropeo")
                    nc.vector.tensor_tensor(t2, t0, t1, mybir.AluOpType.add)
                    # duplicated into both partition halves for the row-tiled
                    # rope matmuls (even skt uses rows 0-63, odd rows 64-127)
                    qpt = spool.tile([128, 512], mm_dt, tag="qpe_h%d" % h, bufs=1)
                    nc.vector.tensor_tensor(qpt[0:64, :], t2, rstd_t[0:64, :],
                                            mybir.AluOpType.mult)
                    nc.sync.dma_start(out=qpt[64:128, :], in_=qpt[0:64, :])
                    qpe_t[h] = qpt

                n_skt = 4 * (sqc + 1)
                ctx_sb = spool.tile([D_V, HPC, 512], mm_dt, tag="ctx", bufs=2)
                for h in range(HPC):
                    sum_acc = psums.tile([1, 512], F32, tag="p_sum", name="sum_acc")
                    ctx_acc = pctx.tile([D_V, 512], F32, tag="p_ctx")
                    pending = None   # software pipeline: exp tile awaiting PV
                    for skp in range(n_skt // 2):
                        scs = []
                        for half in range(2):
                            skt = 2 * skp + half
                            sc = pscore.tile([128, 512], F32, tag="p_sc", name="sc")
                            nc.tensor.matmul(
                                sc,
                                kn_sb[:, h, skt // 4,
                                      (skt % 4) * 128:(skt % 4) * 128 + 128],
                                qn_t[h], start=True, stop=False,
                                skip_group_check=True)
                            scs.append(sc)
                        for half in range(2):
                            nc.tensor.matmul(
                                scs[half],
                                kpe2_sb[half * 64:(half + 1) * 64, skp, :],
                                qpe_t[h][half * 64:(half + 1) * 64, :],
                                start=False, stop=True, skip_group_check=True,
                                tile_position=(half * 64, 0))
                        for half in range(2):
                            skt = 2 * skp + half
                            ex = spool.tile([128, 512], mm_dt,
                                            tag="exp%d" % (skt % 3), bufs=2)
                            nc.scalar.activation(out=ex, in_=scs[half],
                                                 func=mybir.ActivationFunctionType.Exp,
                                                 scale=SCALE)
                            d = skt - 4 * sqc
                            if d >= 0:
                                nc.vector.tensor_tensor(ex, ex, mask_sb[:, d, :],
                                                        mybir.AluOpType.mult)
                            if pending is not None:
                                pex, pskt = pending
                                pd = pskt - 4 * sqc
                                c0 = 128 * pd if pd > 0 else 0
                                nc.tensor.matmul(sum_acc[:, c0:], ones_sb,
                                                 pex[:, c0:],
                                                 start=(pskt == 0), stop=False,
                                                 skip_group_check=True)
                                nc.tensor.matmul(ctx_acc[:, c0:],
                                                 v_sb[:, pskt,
                                                      h * D_V:(h + 1) * D_V],
                                                 pex[:, c0:],
                                                 start=(pskt == 0), stop=False,
                                                 skip_group_check=True)
                            pending = (ex, skt)
                    pex, pskt = pending
                    pd = pskt - 4 * sqc
                    c0 = 128 * pd if pd > 0 else 0
                    nc.tensor.matmul(sum_acc[:, c0:], ones_sb, pex[:, c0:],
                                     start=(pskt == 0), stop=True,
                                     skip_group_check=True)
                    nc.tensor.matmul(ctx_acc[:, c0:],
                                     v_sb[:, pskt, h * D_V:(h + 1) * D_V],
                                     pex[:, c0:], start=(pskt == 0), stop=True,
                                     skip_group_check=True)
                    # 1/sum via exp(-ln(sum)) on ScalarE
                    ls = spool.tile([1, 512], F32, tag="lsum")
                    nc.scalar.activation(out=ls, in_=sum_acc,
                                         func=mybir.ActivationFunctionType.Ln)
                    rc = spool.tile([1, 512], F32, tag="recip1")
                    nc.scalar.activation(out=rc, in_=ls, scale=-1.0,
                                         func=mybir.ActivationFunctionType.Exp)
                    rb = spool.tile([128, 512], F32, tag="recip_bc")
                    nc.gpsimd.partition_broadcast(rb, rc)
                    nc.vector.tensor_tensor(ctx_sb[:, h, :], ctx_acc, rb,
                                            mybir.AluOpType.mult)

                # ---- output projection for this seq chunk ----
                for hidc in range(HID // 128):
                    owt = wpool.tile([D_V, HPC, 128], mm_dt, tag="w_o")
                    nc.sync.dma_start(
                        out=owt, in_=ow.ap()[:, :, hidc * 128:(hidc + 1) * 128])
                    acc = ppool.tile([128, 512], F32, tag="p_a", name="acc_o")
                    for h in range(HPC):
                        nc.tensor.matmul(acc, owt[:, h, :], ctx_sb[:, h, :],
                                         start=(h == 0), stop=(h == HPC - 1))
                    o = spool.tile([128, 512], mm_dt, tag="oout")
                    nc.vector.tensor_copy(o, acc)
                    nc.gpsimd.dma_start(
                        out=out_t.ap()[hidc * 128:(hidc + 1) * 128,
                                       sqc * 512:(sqc + 1) * 512],
                        in_=o)

    nc.compile()
    return nc


# ------------------------------------------------------------- host side --
def _rope_tables():
    inv_freq = 1.0 / (ROPE_THETA ** (np.arange(0, D_ROPE, 2, dtype=np.float64) / D_ROPE))
    t = np.arange(S, dtype=np.float64)
    freqs = np.outer(t, inv_freq)                    # [S, 32]
    emb = np.concatenate([freqs, freqs], axis=-1)    # [S, 64]
    return (np.cos(emb).astype(np.float32).T.copy(),
            np.sin(emb).astype(np.float32).T.copy())  # [64, S]


_E_PERM = np.concatenate([np.arange(0, D_ROPE, 2), np.arange(1, D_ROPE, 2)])


def _rope_expand(Wpe):
    """[n, 64] rope weight cols -> [n, 128]: [even/odd-reordered | rot-half signed]."""
    Y = Wpe[:, _E_PERM]
    R = np.concatenate([-Y[:, D_ROPE // 2:], Y[:, :D_ROPE // 2]], axis=1)
    return np.concatenate([Y, R], axis=1)


def _chunk_rows(a, p=128):
    """[R, C] -> [p, R//p, C] grouping rows into chunks of p."""
    R, Cs = a.shape[0], a.shape[1:]
    return np.ascontiguousarray(a.reshape(R // p, p, *Cs).transpose(
        1, 0, *range(2, a.ndim + 1)))


def _prep_inputs(hidden_states, w_dq, q_a_ln_w, w_uq, kv_a_w, kv_a_ln_w, kv_b_w, o_w):
    bf = ml_dtypes.bfloat16
    s_loc = S // 4
    cosT, sinT = _rope_tables()

    wuq_eff = (np.asarray(q_a_ln_w)[:, None] * np.asarray(w_uq)).reshape(Q_LORA, H, D_Q)
    head_blocks = []
    for h in range(H):
        head_blocks.append(np.concatenate(
            [wuq_eff[:, h, :D_NOPE], _rope_expand(wuq_eff[:, h, D_NOPE:])], axis=1))
    wuq_x = np.stack(head_blocks, axis=1)            # [1536, 16, 256]

    kv_a = np.asarray(kv_a_w)
    wkva_x = np.concatenate([kv_a[:, :KV_LORA], _rope_expand(kv_a[:, KV_LORA:])],
                            axis=1).astype(bf)       # [2048, 640]
    wkva_p = _chunk_rows(wkva_x)                     # [128, 16, 640]
    wkvb_eff = (np.asarray(kv_a_ln_w)[:, None] * np.asarray(kv_b_w)).reshape(KV_LORA, H, 256)
    ow_r = np.asarray(o_w).reshape(H, D_V, HID)

    c_idx = np.arange(512)[None, :]
    r_idx = np.arange(128)[:, None]
    masks = np.stack([(c_idx >= 128 * dd + r_idx) for dd in range(4)],
                     axis=1).astype(bf)              # [128, 4, 512]

    wdq_b = np.asarray(w_dq).astype(bf)
    hs = np.asarray(hidden_states)

    in_maps = []
    for c in range(N_CORES):
        b, hg = c // 4, c % 4
        s0 = 512 * hg
        xt_full = np.ascontiguousarray(hs[b].T).astype(bf)
        wuq_c = wuq_x[:, HPC * hg: HPC * (hg + 1), :].reshape(
            Q_LORA, HPC * 256).astype(bf)
        wkvb_c = wkvb_eff[:, HPC * hg: HPC * (hg + 1), :].astype(bf)
        in_maps.append({
            "xt": xt_full,
            "xt_loc": np.ascontiguousarray(xt_full[:, s0:s0 + s_loc]),
            "wdq": wdq_b,
            "wuq": _chunk_rows(wuq_c),               # [128, 12, 1024]
            "wkva": wkva_p,
            "wkvb": _chunk_rows(wkvb_c),             # [128, 4, 4, 256]
            "ow": np.ascontiguousarray(
                ow_r[HPC * hg: HPC * (hg + 1)].transpose(1, 0, 2)).astype(bf),
            "cos_f": cosT.astype(bf),
            "sin_f": sinT.astype(bf),
            "masks": masks,
        })
    return in_maps


def _postprocess(results):
    out = np.empty((B, S, HID), dtype=np.float32)
    for b in range(B):
        acc = results[4 * b]["out_t"].astype(np.float32)
        for c in GROUPS[b][1:]:
            acc = acc + results[c]["out_t"].astype(np.float32)
        out[b] = acc.T
    return out


def kernel(**inputs):
    key = (str(MM_DT),)
    if key not in _CACHE:
        _CACHE[key] = build_kernel(MM_DT)
    nc = _CACHE[key]
    in_maps = _prep_inputs(**inputs)
    r = run_bass_kernel_spmd(nc, in_maps, core_ids=list(range(N_CORES)))
    return _postprocess(r.results)


# revision 19
# speedup vs baseline: 1.2057x; 1.0919x over previous
"""DeepseekV2 MLA attention prefill kernel for 8 Trainium2 NeuronCores.

Sharding: 2-way data-parallel over batch x 4-way tensor-parallel over heads
(4 heads per core).  The raw q down-projection (+ rstd of its RMS norm) is
computed on an S/4 slice per core and exchanged with one in-group AllGather;
the RMS normalization is folded into the q up-projection output after the
gather.  The compressed-KV path is replicated at full S on every core and
computed while the gather is in flight.  Per-head up-projections, attention
and the output projection are computed locally; o_proj partial sums are
reduced on the host during unsharding.

Key scheduling/efficiency points (v4):
 - score matmuls run in fp8e4 DoubleRow: the two 128-deep k-subtiles are
   [k_nope | (k_pe ; zeros)], so one PE pass per 128x512 score block covers
   the full 192-dim contraction (rope included); q/k packs are built by the
   DVE/DMA on the side.  Everything else stays bf16 (fp8 there fails the
   2e-2 tolerance; scores measured 1.0e-2 in emulation).
 - the collective lives alone on the gpsimd queue; RMS rstd broadcasts are
   done by a K=1 PE matmul against a ones row so the kv-norm never blocks
   behind the 90us gather.
 - DMA priority at startup: only the q-down critical stream (xt_loc + wdq)
   is issued first; all other weights follow it on the same queue.
 - exp is evaluated over [128, 1024] pairs of score banks (halves ScalarE
   instruction overhead); attention context is evacuated raw and the
   softmax 1/sum is applied during a later DVE pass, so the single-bank
   ctx accumulator frees immediately at head boundaries.
 - PV and row-sum matmuls restrict their free dim on diagonal blocks.

Layouts: activations are feature-major ([D, S]); scores are computed
transposed ([s_k, s_q]) so PV needs no transposes.  RoPE uses host-side
permuted/sign-folded weight columns.  PSUM accumulation fp32 throughout.
"""
import sys
sys.path.insert(0, "/opt/trn_rl_repo")

import math
import numpy as np
import ml_dtypes

import concourse.bass as bass
import concourse.tile as tile
from concourse import bacc, mybir
from concourse.bass_utils import run_bass_kernel_spmd

# ---- problem constants (hardcoded; kernel.py must be self-contained) ----
B, S, HID, H = 2, 2048, 2048, 16
Q_LORA, KV_LORA = 1536, 512
D_NOPE, D_ROPE, D_V = 128, 64, 128
D_Q = D_NOPE + D_ROPE
EPS = 1e-6
ROPE_THETA = 10000.0
N_CORES = 8
HPC = 4                      # heads per core
GROUPS = [[0, 1, 2, 3], [4, 5, 6, 7]]

KC = HID // 128              # 16
QC = Q_LORA // 128           # 12
VC = KV_LORA // 128          # 4
NSK = S // 128               # 16 key blocks

F32 = mybir.dt.float32
BF16 = mybir.dt.bfloat16
F8 = mybir.dt.float8e4
MM_DT = BF16
DR = mybir.MatmulPerfMode.DoubleRow

SCALE = 1.0 / math.sqrt(D_Q)

_CACHE = {}


# ---------------------------------------------------------------- builder --
def build_kernel(mm_dt=MM_DT):
    s_loc = S // 4

    nc = bacc.Bacc("TRN2", target_bir_lowering=False, debug=False,
                   num_devices=N_CORES)

    xt = nc.dram_tensor("xt", [HID, S], mm_dt, kind="ExternalInput")
    xt_loc = nc.dram_tensor("xt_loc", [HID, s_loc], mm_dt, kind="ExternalInput")
    wdq = nc.dram_tensor("wdq", [HID, Q_LORA], mm_dt, kind="ExternalInput")
    wuq = nc.dram_tensor("wuq", [128, QC, HPC * 256], mm_dt, kind="ExternalInput")
    wkva = nc.dram_tensor("wkva", [128, KC, KV_LORA + 2 * D_ROPE], mm_dt,
                          kind="ExternalInput")
    wkvb = nc.dram_tensor("wkvb", [128, VC, HPC, 256], mm_dt, kind="ExternalInput")
    ow = nc.dram_tensor("ow", [D_V, HPC, HID], mm_dt, kind="ExternalInput")
    cos_f = nc.dram_tensor("cos_f", [D_ROPE, S], mm_dt, kind="ExternalInput")
    sin_f = nc.dram_tensor("sin_f", [D_ROPE, S], mm_dt, kind="ExternalInput")
    masks = nc.dram_tensor("masks", [128, 4, 512], mm_dt, kind="ExternalInput")
    out_t = nc.dram_tensor("out_t", [HID, S], mm_dt, kind="ExternalOutput")

    with tile.TileContext(nc) as tc:
        import contextlib
        ctx = contextlib.ExitStack()
        with ctx:
            persist = ctx.enter_context(tc.tile_pool(name="persist", bufs=1))
            wpool = ctx.enter_context(tc.tile_pool(name="wpool", bufs=3))
            spool = ctx.enter_context(tc.tile_pool(name="spool", bufs=2))
            xpool = ctx.enter_context(tc.tile_pool(name="xpool", bufs=3))
            # PSUM: ppool 2 + pscore 2x2banks + pctx 1 + psums 1 = 8 banks
            ppool = ctx.enter_context(tc.tile_pool(name="ppool", bufs=2, space="PSUM"))
            pscore = ctx.enter_context(tc.tile_pool(name="pscore", bufs=2, space="PSUM"))
            pctx = ctx.enter_context(tc.tile_pool(name="pctx", bufs=1, space="PSUM"))
            psums = ctx.enter_context(tc.tile_pool(name="psums", bufs=1, space="PSUM"))
            dram = ctx.enter_context(tc.tile_pool(name="dram", bufs=1, space="DRAM"))

            ones_sb = persist.tile([128, 1], mm_dt, tag="ones")
            nc.vector.memset(ones_sb, 1.0)
            onesr_sb = persist.tile([1, 128], mm_dt, tag="onesr")
            nc.vector.memset(onesr_sb, 1.0)
            eps_sb = persist.tile([1, 1], F32, tag="eps")
            nc.vector.memset(eps_sb, EPS)

            # fused fp8 key pack: [d(128), h, skt, {nope | rope}, s_k(128)];
            # rope rows 64-127 are zero so the q-side values there are inert
            kf_sb = persist.tile([128, HPC, NSK, 2, 128], F8, tag="kf")
            nc.vector.memset(kf_sb[64:128, :, :, 1, :], 0.0)
            # fp8 q packs: rows 64-127 of the rope subtile are never written,
            # and uninitialized fp8 bytes can decode as NaN (NaN*0=NaN in the
            # PE), so zero them once up front.
            qf_t = {}
            for h in range(HPC):
                qf_t[h] = persist.tile([128, 2, 512], F8, tag="qf_h%d" % h,
                                       name="qf%d" % h)
                nc.vector.memset(qf_t[h][64:128, 1, :], 0.0)

            # ---- q-down critical DMA stream first: xt_loc + wdq ----
            xtl_sb = persist.tile([128, KC, 512], mm_dt, tag="xtl")
            mgs = [list(range(g, g + 4)) for g in range(0, QC, 4)]
            wdq_t = {}
            for k in range(KC):
                nc.sync.dma_start(out=xtl_sb[:, k, :],
                                  in_=xt_loc.ap()[k * 128:(k + 1) * 128, :])
                wt = wpool.tile([128, 512], mm_dt, tag="w_s1", bufs=6)
                nc.sync.dma_start(
                    out=wt, in_=wdq.ap()[k * 128:(k + 1) * 128, 0:512])
                wdq_t[(0, k)] = wt
            for gi in range(1, 3):
                for k in range(KC):
                    wt = wpool.tile([128, 512], mm_dt, tag="w_s1", bufs=6)
                    nc.sync.dma_start(
                        out=wt,
                        in_=wdq.ap()[k * 128:(k + 1) * 128,
                                     gi * 512:(gi + 1) * 512])
                    wdq_t[(gi, k)] = wt

            # gather buffers (DRAM): 12 raw q-down chunks + broadcast rstd
            GR = Q_LORA + 128
            g_in = dram.tile([GR, s_loc], mm_dt)
            g_out = dram.tile([4 * GR, 512], mm_dt)

            # ---- stage 1a: q down-proj (raw) + RMS stats on local slice ----
            ssq_q = psums.tile([1, 512], F32, tag="p_sum", name="ssq_q")
            for gi, mg in enumerate(mgs):
                a2 = [pscore.tile([128, 2, 512], F32, tag="p_sc2", name="acc2")
                      for _ in range(2)]
                accs = {m: a2[j // 2][:, j % 2] for j, m in enumerate(mg)}
                for k in range(KC):
                    wt = wdq_t.pop((gi, k))
                    for j, m in enumerate(mg):
                        nc.tensor.matmul(
                            accs[m], wt[:, j * 128:(j + 1) * 128], xtl_sb[:, k, :],
                            start=(k == 0), stop=(k == KC - 1))
                for m in mg:
                    sq = spool.tile([128, 512], mm_dt, tag="sq")
                    nc.scalar.activation(out=sq, in_=accs[m],
                                         func=mybir.ActivationFunctionType.Square)
                    nc.tensor.matmul(ssq_q, ones_sb, sq,
                                     start=(m == 0), stop=(m == QC - 1),
                                     skip_group_check=True)
                    r = spool.tile([128, 512], mm_dt, tag="qdout%d" % (m % 4))
                    nc.vector.tensor_copy(r, accs[m])
                    nc.sync.dma_start(out=g_in[m * 128:(m + 1) * 128, :], in_=r)
            sd = spool.tile([1, 512], F32, tag="sdn")
            nc.scalar.activation(out=sd, in_=ssq_q,
                                 func=mybir.ActivationFunctionType.Sqrt,
                                 bias=eps_sb, scale=1.0 / Q_LORA)
            rstd = spool.tile([1, 512], F32, tag="rstdn")
            nc.vector.reciprocal(rstd, sd)
            rstd_b = spool.tile([1, 512], mm_dt, tag="rstdb")
            nc.vector.tensor_copy(rstd_b, rstd)
            # partition-broadcast via K=1 matmul (gpsimd only has the gather)
            rbc_ps = ppool.tile([128, 512], F32, tag="p_a", name="rbc_q")
            nc.tensor.matmul(rbc_ps, onesr_sb, rstd_b, start=True, stop=True)
            rstd_bcb = spool.tile([128, 512], mm_dt, tag="rstd_bcb_q", bufs=1)
            nc.vector.tensor_copy(rstd_bcb, rbc_ps)
            nc.sync.dma_start(out=g_in[Q_LORA:GR, :], in_=rstd_bcb)

            # ---- stage 1b: AllGather within batch groups (gpsimd queue) ----
            nc.gpsimd.collective_compute(
                "AllGather", mybir.AluOpType.bypass,
                replica_groups=GROUPS,
                ins=[g_in.opt()], outs=[g_out.opt()])

            # remaining weights (sync queue, behind the q-down stream)
            wkva_sb = persist.tile([128, KC, KV_LORA + 2 * D_ROPE], mm_dt, tag="wkva")
            nc.sync.dma_start(out=wkva_sb, in_=wkva.ap())
            wkvb_sb = persist.tile([128, VC, HPC, 256], mm_dt, tag="wkvb")
            nc.sync.dma_start(out=wkvb_sb, in_=wkvb.ap())
            mask_sb = persist.tile([128, 4, 512], mm_dt, tag="masks")
            nc.sync.dma_start(out=mask_sb, in_=masks.ap())
            cosf_sb = persist.tile([D_ROPE, 4, 512], mm_dt, tag="cosf")
            sinf_sb = persist.tile([D_ROPE, 4, 512], mm_dt, tag="sinf")
            nc.sync.dma_start(out=cosf_sb,
                              in_=cos_f.ap().rearrange("d (c n) -> d c n", c=4))
            nc.sync.dma_start(out=sinf_sb,
                              in_=sin_f.ap().rearrange("d (c n) -> d c n", c=4))

            # ---- stage 1c (overlaps gather): compressed KV at full S ----
            ckv_sb = persist.tile([128, VC, 4, 512], mm_dt, tag="ckv")
            for nch in range(4):
                ssq = psums.tile([1, 512], F32, tag="p_sum", name="ssq_kv")
                a2 = [pscore.tile([128, 2, 512], F32, tag="p_sc2", name="acc2")
                      for _ in range(2)]
                accs = {m: a2[m // 2][:, m % 2] for m in range(4)}
                acc_r = ppool.tile([128, 512], F32, tag="p_a", name="acc_rope")
                accs[4] = acc_r
                for k in range(KC):
                    xtt = xpool.tile([128, 512], mm_dt, tag="xt_s")
                    nc.sync.dma_start(
                        out=xtt,
                        in_=xt.ap()[k * 128:(k + 1) * 128,
                                    nch * 512:(nch + 1) * 512])
                    for m in range(5):
                        nc.tensor.matmul(
                            accs[m], wkva_sb[:, k, m * 128:(m + 1) * 128], xtt,
                            start=(k == 0), stop=(k == KC - 1))
                raw = []
                for m in range(4):
                    sq = spool.tile([128, 512], mm_dt, tag="sq")
                    nc.scalar.activation(out=sq, in_=accs[m],
                                         func=mybir.ActivationFunctionType.Square)
                    nc.tensor.matmul(ssq, ones_sb, sq,
                                     start=(m == 0), stop=(m == 3),
                                     skip_group_check=True)
                    r = spool.tile([128, 512], mm_dt, tag="kvraw%d" % m, bufs=1)
                    nc.vector.tensor_copy(r, accs[m])
                    raw.append((m, r))
                # rope chunk [E(64) | R(64)] -> k_pe (fp8), fanned into kf
                t0 = spool.tile([D_ROPE, 512], mm_dt, tag="ropet0")
                t1 = spool.tile([D_ROPE, 512], mm_dt, tag="ropet1")
                nc.vector.tensor_tensor(t0, acc_r[0:D_ROPE, :],
                                        cosf_sb[:, nch, :], mybir.AluOpType.mult)
                nc.vector.tensor_tensor(t1, acc_r[D_ROPE:2 * D_ROPE, :],
                                        sinf_sb[:, nch, :], mybir.AluOpType.mult)
                pe8 = spool.tile([D_ROPE, 512], F8, tag="ropeo8")
                nc.vector.tensor_tensor(pe8, t0, t1, mybir.AluOpType.add)
                for h in range(HPC):
                    nc.sync.dma_start(
                        out=kf_sb[0:64, h, 4 * nch:4 * nch + 4, 1, :],
                        in_=pe8)
                sd2 = spool.tile([1, 512], F32, tag="sdn")
                nc.scalar.activation(out=sd2, in_=ssq,
                                     func=mybir.ActivationFunctionType.Sqrt,
                                     bias=eps_sb, scale=1.0 / KV_LORA)
                rstd2 = spool.tile([1, 512], F32, tag="rstdn")
                nc.vector.reciprocal(rstd2, sd2)
                rstd2_b = spool.tile([1, 512], mm_dt, tag="rstdb")
                nc.vector.tensor_copy(rstd2_b, rstd2)
                rbc2 = ppool.tile([128, 512], F32, tag="p_a", name="rbc_kv")
                nc.tensor.matmul(rbc2, onesr_sb, rstd2_b, start=True, stop=True)
                for m, r in raw:
                    nc.vector.tensor_tensor(ckv_sb[:, m, nch, :], r, rbc2,
                                            mybir.AluOpType.mult)

            # ---- stage 2a: decompress KV (full S, local heads) ----
            for h in range(HPC):
                for skc in range(4):
                    acc = ppool.tile([128, 512], F32, tag="p_a", name="acc_kn")
                    for k in range(VC):
                        nc.tensor.matmul(acc, wkvb_sb[:, k, h, 0:128],
                                         ckv_sb[:, k, skc, :],
                                         start=(k == 0), stop=(k == VC - 1))
                    nc.vector.tensor_copy(
                        kf_sb[:, h, 4 * skc:4 * skc + 4, 0, :], acc)

            v_sb = persist.tile([128, NSK, HPC * D_V], mm_dt, tag="v")
            for skt in range(NSK):
                acc = ppool.tile([128, 512], F32, tag="p_a", name="acc_v")
                for k in range(VC):
                    nc.tensor.matmul(
                        acc,
                        ckv_sb[:, k, skt // 4, (skt % 4) * 128:(skt % 4) * 128 + 128],
                        wkvb_sb[:, k, :, 128:256],
                        start=(k == 0), stop=(k == VC - 1))
                nc.vector.tensor_copy(v_sb[:, skt, :], acc)

            # ---- stage 2b/2c/2d: per-seq-chunk q up-proj, attention, o ----
            for sqc in range(4):
                qnorm_t = [spool.tile([128, 512], mm_dt, tag="qn_stream%d" % (k % 4),
                                      name="qnorm_t", bufs=3) for k in range(QC)]
                for k in range(QC):
                    nc.sync.dma_start(
                        out=qnorm_t[k],
                        in_=g_out[sqc * GR + k * 128: sqc * GR + (k + 1) * 128, :])
                rstd_t = spool.tile([128, 512], mm_dt, tag="rstd_t", bufs=2)
                nc.sync.dma_start(out=rstd_t,
                                  in_=g_out[sqc * GR + Q_LORA: (sqc + 1) * GR, :])
                for h in range(HPC):
                    wts = []
                    for k in range(QC):
                        wt = wpool.tile([128, 256], mm_dt, tag="w_uq")
                        nc.sync.dma_start(
                            out=wt, in_=wuq.ap()[:, k, h * 256:(h + 1) * 256])
                        wts.append(wt)
                    acc2 = pscore.tile([128, 2, 512], F32, tag="p_sc2", name="acc_qup")
                    for k in range(QC):
                        for j in range(2):
                            nc.tensor.matmul(
                                acc2[:, j], wts[k][:, j * 128:(j + 1) * 128],
                                qnorm_t[k],
                                start=(k == 0), stop=(k == QC - 1))
                    # fp8 q pack [nope | rope]; RMS rstd folded in here
                    qf = qf_t[h]
                    nc.vector.tensor_tensor(qf[:, 0, :], acc2[:, 0], rstd_t,
                                            mybir.AluOpType.mult)
                    t0 = spool.tile([D_ROPE, 512], mm_dt, tag="ropet0")
                    t1 = spool.tile([D_ROPE, 512], mm_dt, tag="ropet1")
                    nc.vector.tensor_tensor(t0, acc2[0:D_ROPE, 1], cosf_sb[:, sqc, :],
                                            mybir.AluOpType.mult)
                    nc.vector.tensor_tensor(t1, acc2[D_ROPE:2 * D_ROPE, 1],
                                            sinf_sb[:, sqc, :], mybir.AluOpType.mult)
                    t2 = spool.tile([D_ROPE, 512], mm_dt, tag="ropeo")
                    nc.vector.tensor_tensor(t2, t0, t1, mybir.AluOpType.add)
                    nc.vector.tensor_tensor(qf[0:64, 1, :], t2, rstd_t[0:64, :],
                                            mybir.AluOpType.mult)
                    qf_t[h] = qf

                n_skt = 4 * (sqc + 1)
                ctx_sb = spool.tile([D_V, HPC, 512], mm_dt, tag="ctx", bufs=2)
                for h in range(HPC):
                    sum_acc = psums.tile([1, 512], F32, tag="p_sum", name="sum_acc")
                    ctx_acc = pctx.tile([D_V, 512], F32, tag="p_ctx")

                    def drain(pex2, pskp):
                        for half in range(2):
                            skt = 2 * pskp + half
                            pd = skt - 4 * sqc
                            c0 = 128 * pd if pd > 0 else 0
                            pex = pex2[:, half]
                            nc.tensor.matmul(sum_acc[:, c0:], ones_sb,
                                             pex[:, c0:],
                                             start=(skt == 0),
                                             stop=(skt == n_skt - 1),
                                             skip_group_check=True)
                            nc.tensor.matmul(ctx_acc[:, c0:],
                                             v_sb[:, skt, h * D_V:(h + 1) * D_V],
                                             pex[:, c0:],
                                             start=(skt == 0),
                                             stop=(skt == n_skt - 1),
                                             skip_group_check=True)

                    pending = None   # software pipeline: exp pair awaiting PV
                    for skp in range(n_skt // 2):
                        sc2 = pscore.tile([128, 2, 512], F32, tag="p_sc2",
                                          name="sc2")
                        for half in range(2):
                            skt = 2 * skp + half
                            nc.tensor.matmul(
                                sc2[:, half], kf_sb[:, h, skt], qf_t[h],
                                start=True, stop=True, perf_mode=DR,
                                skip_group_check=True)
                        ex2 = spool.tile([128, 2, 512], mm_dt,
                                         tag="exp%d" % (skp % 2), bufs=2)
                        nc.scalar.activation(out=ex2, in_=sc2,
                                             func=mybir.ActivationFunctionType.Exp,
                                             scale=SCALE)
                        d0 = 2 * skp - 4 * sqc
                        if d0 >= 0:
                            nc.vector.tensor_tensor(ex2, ex2,
                                                    mask_sb[:, d0:d0 + 2, :],
                                                    mybir.AluOpType.mult)
                        if pending is not None:
                            drain(*pending)
                        pending = (ex2, skp)
                    drain(*pending)
                    # raw evacuation frees the single ctx bank immediately
                    ctxr = spool.tile([D_V, 512], mm_dt, tag="ctxr%d" % h, bufs=1)
                    nc.vector.tensor_copy(ctxr, ctx_acc)
                    # 1/sum via exp(-ln(sum)) on ScalarE
                    ls = spool.tile([1, 512], F32, tag="lsum")
                    nc.scalar.activation(out=ls, in_=sum_acc,
                                         func=mybir.ActivationFunctionType.Ln)
                    rc = spool.tile([1, 512], F32, tag="recip1")
                    nc.scalar.activation(out=rc, in_=ls, scale=-1.0,
                                         func=mybir.ActivationFunctionType.Exp)
                    rb = spool.tile([128, 512], F32, tag="recip_bc")
                    nc.gpsimd.partition_broadcast(rb, rc)
                    nc.vector.tensor_tensor(ctx_sb[:, h, :], ctxr, rb,
                                            mybir.AluOpType.mult)

                # ---- output projection for this seq chunk ----
                for hidc in range(HID // 128):
                    owt = wpool.tile([D_V, HPC, 128], mm_dt, tag="w_o")
                    nc.sync.dma_start(
                        out=owt, in_=ow.ap()[:, :, hidc * 128:(hidc + 1) * 128])
                    acc = ppool.tile([128, 512], F32, tag="p_a", name="acc_o")
                    for h in range(HPC):
                        nc.tensor.matmul(acc, owt[:, h, :], ctx_sb[:, h, :],
                                         start=(h == 0), stop=(h == HPC - 1))
                    o = spool.tile([128, 512], mm_dt, tag="oout")
                    nc.vector.tensor_copy(o, acc)
                    nc.gpsimd.dma_start(
                        out=out_t.ap()[hidc * 128:(hidc + 1) * 128,
                                       sqc * 512:(sqc + 1) * 512],
                        in_=o)

    nc.compile()
    return nc


# ------------------------------------------------------------- host side --
def _rope_tables():
    inv_freq = 1.0 / (ROPE_THETA ** (np.arange(0, D_ROPE, 2, dtype=np.float64) / D_ROPE))
    t = np.arange(S, dtype=np.float64)
    freqs = np.outer(t, inv_freq)                    # [S, 32]
    emb = np.concatenate([freqs, freqs], axis=-1)    # [S, 64]
    return (np.cos(emb).astype(np.float32).T.copy(),
            np.sin(emb).astype(np.float32).T.copy())  # [64, S]


_E_PERM = np.concatenate([np.arange(0, D_ROPE, 2), np.arange(1, D_ROPE, 2)])


def _rope_expand(Wpe):
    """[n, 64] rope weight cols -> [n, 128]: [even/odd-reordered | rot-half signed]."""
    Y = Wpe[:, _E_PERM]
    R = np.concatenate([-Y[:, D_ROPE // 2:], Y[:, :D_ROPE // 2]], axis=1)
    return np.concatenate([Y, R], axis=1)


def _chunk_rows(a, p=128):
    """[R, ...] -> [p, R//p, ...] grouping rows into chunks of p."""
    R, Cs = a.shape[0], a.shape[1:]
    return np.ascontiguousarray(a.reshape(R // p, p, *Cs).transpose(
        1, 0, *range(2, a.ndim + 1)))


def _prep_inputs(hidden_states, w_dq, q_a_ln_w, w_uq, kv_a_w, kv_a_ln_w, kv_b_w, o_w):
    bf = ml_dtypes.bfloat16
    s_loc = S // 4
    cosT, sinT = _rope_tables()

    wuq_eff = (np.asarray(q_a_ln_w)[:, None] * np.asarray(w_uq)).reshape(Q_LORA, H, D_Q)
    head_blocks = []
    for h in range(H):
        head_blocks.append(np.concatenate(
            [wuq_eff[:, h, :D_NOPE], _rope_expand(wuq_eff[:, h, D_NOPE:])], axis=1))
    wuq_x = np.stack(head_blocks, axis=1)            # [1536, 16, 256]

    kv_a = np.asarray(kv_a_w)
    wkva_x = np.concatenate([kv_a[:, :KV_LORA], _rope_expand(kv_a[:, KV_LORA:])],
                            axis=1).astype(bf)       # [2048, 640]
    wkva_p = _chunk_rows(wkva_x)                     # [128, 16, 640]
    wkvb_eff = (np.asarray(kv_a_ln_w)[:, None] * np.asarray(kv_b_w)).reshape(KV_LORA, H, 256)
    ow_r = np.asarray(o_w).reshape(H, D_V, HID)

    c_idx = np.arange(512)[None, :]
    r_idx = np.arange(128)[:, None]
    masks = np.stack([(c_idx >= 128 * dd + r_idx) for dd in range(4)],
                     axis=1).astype(bf)              # [128, 4, 512]

    wdq_b = np.asarray(w_dq).astype(bf)
    hs = np.asarray(hidden_states)

    in_maps = []
    for c in range(N_CORES):
        b, hg = c // 4, c % 4
        s0 = 512 * hg
        xt_full = np.ascontiguousarray(hs[b].T).astype(bf)
        wuq_c = wuq_x[:, HPC * hg: HPC * (hg + 1), :].reshape(
            Q_LORA, HPC * 256).astype(bf)
        wkvb_c = wkvb_eff[:, HPC * hg: HPC * (hg + 1), :].astype(bf)
        in_maps.append({
            "xt": xt_full,
            "xt_loc": np.ascontiguousarray(xt_full[:, s0:s0 + s_loc]),
            "wdq": wdq_b,
            "wuq": _chunk_rows(wuq_c),               # [128, 12, 1024]
            "wkva": wkva_p,
            "wkvb": _chunk_rows(wkvb_c),             # [128, 4, 4, 256]
            "ow": np.ascontiguousarray(
                ow_r[HPC * hg: HPC * (hg + 1)].transpose(1, 0, 2)).astype(bf),
            "cos_f": cosT.astype(bf),
            "sin_f": sinT.astype(bf),
            "masks": masks,
        })
    return in_maps


def _postprocess(results):
    out = np.empty((B, S, HID), dtype=np.float32)
    for b in range(B):
        acc = results[4 * b]["out_t"].astype(np.float32)
        for c in GROUPS[b][1:]:
            acc = acc + results[c]["out_t"].astype(np.float32)
        out[b] = acc.T
    return out


def kernel(**inputs):
    key = (str(MM_DT),)
    if key not in _CACHE:
        _CACHE[key] = build_kernel(MM_DT)
    nc = _CACHE[key]
    in_maps = _prep_inputs(**inputs)
    r = run_bass_kernel_spmd(nc, in_maps, core_ids=list(range(N_CORES)))
    return _postprocess(r.results)
